# revision 11
# baseline (speedup 1.0000x reference)
"""Trainium2 Bass kernel for nn_ExpandOperator (banded scatter of a linear projection).

Reference semantics:
    pred = x @ W.T + b                      # (B, S, 2048)
    pred = pred.reshape(B, S, 64, 32)
    out[b, t, (t+s) % S, d] = pred[b, t, s, d]   # rest of out is zeros
    out shape: (B, S, S, 32) fp32  == 1 GiB

Sharding: 8 cores = (batch b in {0,1}) x (512-row seq chunk cc in {0..3}).
Each core computes pred for its 512 rows and writes its 128 MiB output slice.

Core-invariant SPMD trick: every core runs the identical program pretending its
rows are t = 0..511, so the scatter band sits on a fixed diagonal with no
wraparound.  The host rotates each core's block along the S axis by 512*cc
when unsharding (pure block memcpy).

Diagonal layout: the per-core output is declared as (512, 65568) where row t is
[2048-float band | 63520 floats of zeros].  Row-major linearization of this
buffer equals the true (512, 2048, 32) slice (band t lives at flat offset
65536*t + 32*t = 65568*t) plus a 64 KiB tail pad that the host drops.  Band and
gap writes are uniform strided DMAs covering every true output byte exactly
once - no overlapping writes, no ordering hazards.

This walrus build only leaves room for ONE sync-wait per compute instruction,
so everything a matmul/copy depends on must arrive through a single semaphore:
W.T, x.T, the bias AND a ones-row are packed into ONE input tensor "wx" loaded
by a single DMA.  The bias is folded into the matmul contraction itself: wx is
padded from 768 to 896 rows (7 K-tiles of 128), with row 768 = [b | 1.0s] and
rows 769..895 = 0, so `x_aug @ Waug.T` = x @ W.T + b with no extra ops.
"""

import numpy as np

import bass_rust
import concourse.bass as bass
import concourse.mybir as mybir
import concourse.tile as tile
from concourse.bass_utils import run_bass_kernel_spmd

F32 = mybir.dt.float32


def _split_multi_waits(nc):
    """Walrus in this toolchain only leaves ONE sync-wait slot per
    instruction.  Tile's tail drain waits on every semaphore lane it used
    (14 here), which fails codegen.  Hoist all-but-one wait of any multi-wait
    instruction into single-wait NOPs on the same engine queue immediately
    before it - semantically identical (same-queue waits execute in order).
    """
    eng_by_type = {
        mybir.EngineType.SP: nc.sync,
        mybir.EngineType.PE: nc.tensor,
        mybir.EngineType.Activation: nc.scalar,
        mybir.EngineType.Pool: nc.gpsimd,
        mybir.EngineType.DVE: nc.vector,
    }
    tail_bb = nc.cur_bb.bb
    for f in nc.m.functions:
        for bb in f.blocks:
            il = bb.instructions
            i = 0
            while i < len(il):
                ins = il[i]
                si = getattr(ins, "sync_info", None)
                if si is not None and len(si.on_wait) > 1:
                    waits = list(si.on_wait)
                    for w in waits[:-1]:
                        nop = eng_by_type[ins.engine].nop(nofuse=True).ins
                        tail_bb.instructions.remove(nop)
                        nop.sync_info = bass_rust.SyncInfo(
                            on_wait=[w], on_update=[])
                        il.insert(i, nop)
                        i += 1
                    ins.sync_info = bass_rust.SyncInfo(
                        on_wait=[waits[-1]], on_update=list(si.on_update))
                i += 1

# Problem shapes (hardcoded per contract).
B = 2
S = 2048
D_IN = 768
MAX_SPAN = 64
SPAN_DIM = 32
N_OUT = MAX_SPAN * SPAN_DIM  # 2048
N_CORES = 8
CHUNKS = 4                   # seq chunks per batch (B * CHUNKS == N_CORES)
ROWS = S // CHUNKS           # 512 rows per core


def build_nc(rows=ROWS, s=S, d_in=D_IN, n_out=N_OUT, span_dim=SPAN_DIM,
             gap_split=8):
    """Build the single-core Bass program (shared by all 8 cores via SPMD).

    Inputs (per core):
      wx : (d_pad, n_out + rows)  [Waug.T | x_aug.T] packed -> one DMA load,
           d_pad = round_up(d_in + 1, 128); row d_in = [b | 1.0s], rest 0.
    Output:
      out: (rows, period) diagonal-layout buffer, period = s*span_dim + span_dim
    """
    row_f = s * span_dim            # true floats per output row
    period = row_f + span_dim       # diagonal period (band marches span_dim/row)
    gap = period - n_out            # zero floats after each band
    assert gap % gap_split == 0
    gw = gap // gap_split           # floats per gap-chunk DMA
    d_pad = -(-(d_in + 1) // 128) * 128
    kt = d_pad // 128               # contraction tiles (incl. bias tile)
    mblk = rows // 128              # 128-row blocks
    nw = min(512, n_out)            # psum chunk width (one fp32 bank)
    nchunk = n_out // nw
    wcols = n_out + rows            # packed free width

    nc = bass.Bass()
    wx = nc.dram_tensor("wx", [d_pad, wcols], F32, kind="ExternalInput")
    out = nc.dram_tensor("out", [rows, period], F32, kind="ExternalOutput")

    wx_r = wx.rearrange("(k p) m -> p k m", p=128)   # (128, kt, wcols)

    with tile.TileContext(nc) as tc:
        with (
            tc.tile_pool(name="const", bufs=1) as cpool,
            tc.tile_pool(name="pred", bufs=mblk) as ppool,
            tc.tile_pool(name="psum", bufs=4, space="PSUM") as pspool,
        ):
            # Zero source tile for the gap writes.
            zt = cpool.tile([128, gw], F32)
            nc.vector.memset(zt[:], 0.0)

            # Gap writes: everything after each band, uniform strided DMAs.
            # These only depend on the memset, so they start immediately.
            for mb in range(mblk):
                rs = mb * 128
                for g in range(gap_split):
                    cs = n_out + g * gw
                    nc.sync.dma_start(out[rs:rs + 128, cs:cs + gw], zt[:])

            # Weights + activations + bias row in one DMA (one semaphore).
            # Issued on the scalar HWDGE ring so it never queues behind the
            # gap stores on the sync ring.
            wx_sb = cpool.tile([128, kt, wcols], F32)
            nc.scalar.dma_start(wx_sb[:], wx_r[:])

            # pred = x @ W.T + b, one 128-row block at a time.
            for mb in range(mblk):
                rs = mb * 128
                pt = ppool.tile([128, n_out], F32)
                for n in range(nchunk):
                    ns = n * nw
                    ps = pspool.tile([128, nw], F32)
                    for k in range(kt):
                        nc.tensor.matmul(
                            ps[:],
                            wx_sb[:, k, n_out + rs:n_out + rs + 128],  # x.T blk
                            wx_sb[:, k, ns:ns + nw],                   # W.T blk
                            start=(k == 0),
                            stop=(k == kt - 1),
                        )
                    # PSUM -> SBUF move (bias already folded into matmul).
                    nc.vector.tensor_copy(pt[:, ns:ns + nw], ps[:])
                # Band write: row t of this block goes to out[t, 0:n_out],
                # which in flat space is the diagonal 65568*t + [0, 2048).
                # Issued via SWDGE (gpsimd) whose lanes are otherwise idle:
                # every instruction here may carry at most ONE sync wait, and
                # on the sync ring this DMA would need a lane-FIFO wait on top
                # of its DVE data wait.
                nc.gpsimd.dma_start(out[rs:rs + 128, 0:n_out], pt[:])

    _split_multi_waits(nc)
    return nc


_CACHE = {}


def _get_nc():
    if "nc" not in _CACHE:
        _CACHE["nc"] = build_nc()
    return _CACHE["nc"]


def make_in_maps(x, W, b):
    """Host-side sharding: per-core packed input dicts."""
    d_pad = -(-(D_IN + 1) // 128) * 128  # 896
    x = x.astype(np.float32, copy=False)
    W = W.astype(np.float32, copy=False)
    b = b.astype(np.float32, copy=False)
    in_maps = []
    for c in range(N_CORES):
        bi, cc = divmod(c, CHUNKS)
        xs = x[bi, cc * ROWS:(cc + 1) * ROWS, :]
        wx_np = np.zeros((d_pad, N_OUT + ROWS), np.float32)
        wx_np[:D_IN, :N_OUT] = W.T
        wx_np[:D_IN, N_OUT:] = xs.T
        wx_np[D_IN, :N_OUT] = b
        wx_np[D_IN, N_OUT:] = 1.0
        in_maps.append({"wx": wx_np})
    return in_maps


def unshard(results):
    """Host-side unsharding: drop tail pad, rotate along S by 512*cc, place."""
    row_f = S * SPAN_DIM
    out = np.empty((B, S, S, SPAN_DIM), np.float32)
    for c in range(N_CORES):
        bi, cc = divmod(c, CHUNKS)
        buf = np.asarray(results[c]["out"])
        local = buf.reshape(-1)[:ROWS * row_f].reshape(ROWS, S, SPAN_DIM)
        sh = cc * ROWS
        blk = out[bi, sh:sh + ROWS]
        if sh:
            blk[:, sh:, :] = local[:, :S - sh, :]
            blk[:, :sh, :] = local[:, S - sh:, :]
        else:
            blk[:, :, :] = local
    return out


def kernel(x, W, b):
    x = np.asarray(x)
    W = np.asarray(W)
    b = np.asarray(b)
    nc = _get_nc()
    res = run_bass_kernel_spmd(nc, make_in_maps(x, W, b),
                               list(range(N_CORES)))
    return unshard(res.results)


# revision 13
# speedup vs baseline: 1.5370x; 1.5370x over previous
"""Trainium2 Bass kernel for nn_ExpandOperator (banded scatter of a linear projection).

Reference semantics:
    pred = x @ W.T + b                      # (B, S, 2048)
    pred = pred.reshape(B, S, 64, 32)
    out[b, t, (t+s) % S, d] = pred[b, t, s, d]   # rest of out is zeros
    out shape: (B, S, S, 32) fp32  == 1 GiB

Sharding: 8 cores = (batch b in {0,1}) x (512-row seq chunk cc in {0..3}).
Each core computes pred for its 512 rows and writes its 128 MiB output slice.

Core-invariant SPMD trick: every core runs the identical program pretending its
rows are t = 0..511, so the scatter band sits on a fixed diagonal with no
wraparound.  The host rotates each core's block along the S axis by 512*cc
when unsharding (pure block memcpy).

Diagonal layout: the per-core output is declared as (512, 65568) where row t is
[2048-float band | 63520 floats of zeros].  Row-major linearization of this
buffer equals the true (512, 2048, 32) slice (band t lives at flat offset
65536*t + 32*t = 65568*t) plus a 64 KiB tail pad that the host drops.  Band and
gap writes are uniform strided DMAs covering every true output byte exactly
once - no overlapping writes, no ordering hazards.

This walrus build only leaves room for ONE sync-wait per compute instruction,
so everything a matmul/copy depends on must arrive through a single semaphore:
W.T, x.T, the bias AND a ones-row are packed into ONE input tensor "wx" loaded
by a single DMA.  The bias is folded into the matmul contraction itself: wx is
padded from 768 to 896 rows (7 K-tiles of 128), with row 768 = [b | 1.0s] and
rows 769..895 = 0, so `x_aug @ Waug.T` = x @ W.T + b with no extra ops.
"""

import numpy as np

import bass_rust
import concourse.bass as bass
import concourse.mybir as mybir
import concourse.tile as tile
from concourse.bass_utils import run_bass_kernel_spmd

F32 = mybir.dt.float32


def _split_multi_waits(nc):
    """Walrus in this toolchain only leaves ONE sync-wait slot per
    instruction.  Tile's tail drain waits on every semaphore lane it used
    (14 here), which fails codegen.  Hoist all-but-one wait of any multi-wait
    instruction into single-wait NOPs on the same engine queue immediately
    before it - semantically identical (same-queue waits execute in order).
    """
    eng_by_type = {
        mybir.EngineType.SP: nc.sync,
        mybir.EngineType.PE: nc.tensor,
        mybir.EngineType.Activation: nc.scalar,
        mybir.EngineType.Pool: nc.gpsimd,
        mybir.EngineType.DVE: nc.vector,
    }
    tail_bb = nc.cur_bb.bb
    for f in nc.m.functions:
        for bb in f.blocks:
            il = bb.instructions
            i = 0
            while i < len(il):
                ins = il[i]
                si = getattr(ins, "sync_info", None)
                if si is not None and len(si.on_wait) > 1:
                    waits = list(si.on_wait)
                    for w in waits[:-1]:
                        nop = eng_by_type[ins.engine].nop(nofuse=True).ins
                        tail_bb.instructions.remove(nop)
                        nop.sync_info = bass_rust.SyncInfo(
                            on_wait=[w], on_update=[])
                        il.insert(i, nop)
                        i += 1
                    ins.sync_info = bass_rust.SyncInfo(
                        on_wait=[waits[-1]], on_update=list(si.on_update))
                i += 1

# Problem shapes (hardcoded per contract).
B = 2
S = 2048
D_IN = 768
MAX_SPAN = 64
SPAN_DIM = 32
N_OUT = MAX_SPAN * SPAN_DIM  # 2048
N_CORES = 8
CHUNKS = 4                   # seq chunks per batch (B * CHUNKS == N_CORES)
ROWS = S // CHUNKS           # 512 rows per core


def build_nc(rows=ROWS, s=S, d_in=D_IN, n_out=N_OUT, span_dim=SPAN_DIM,
             gap_split=8, repeats=1):
    """Build the single-core Bass program (shared by all 8 cores via SPMD).

    Inputs (per core):
      wx : (d_pad, n_out + rows)  [Waug.T | x_aug.T] packed -> one DMA load,
           d_pad = round_up(d_in + 1, 128); row d_in = [b | 1.0s], rest 0.
    Output:
      out: (rows, period) diagonal-layout buffer, period = s*span_dim + span_dim
    """
    row_f = s * span_dim            # true floats per output row
    period = row_f + span_dim       # diagonal period (band marches span_dim/row)
    gap = period - n_out            # zero floats after each band
    assert gap % gap_split == 0
    gw = gap // gap_split           # floats per gap-chunk DMA
    d_pad = -(-(d_in + 1) // 128) * 128
    kt = d_pad // 128               # contraction tiles (incl. bias tile)
    mblk = rows // 128              # 128-row blocks
    nw = min(512, n_out)            # psum chunk width (one fp32 bank)
    nchunk = n_out // nw
    wcols = n_out + rows            # packed free width

    nc = bass.Bass()
    wx = nc.dram_tensor("wx", [d_pad, wcols], F32, kind="ExternalInput")
    out = nc.dram_tensor("out", [rows, period], F32, kind="ExternalOutput")

    wx_r = wx.rearrange("(k p) m -> p k m", p=128)   # (128, kt, wcols)

    with tile.TileContext(nc) as tc:
        with (
            tc.tile_pool(name="const", bufs=1) as cpool,
            tc.tile_pool(name="pred", bufs=mblk) as ppool,
            tc.tile_pool(name="psum", bufs=4, space="PSUM") as pspool,
        ):
            # Zero source tile for the gap writes.
            zt = cpool.tile([128, gw], F32)
            nc.vector.memset(zt[:], 0.0)

            # repeats>1 duplicates the whole body for timing measurements
            # (the dispatch path has a ~650us/call floor that hides the
            # kernel; differencing repeat counts cancels it).
            for _rep in range(repeats):
                # Gap writes: everything after each band, uniform strided
                # DMAs.  These only depend on the memset, so they start
                # immediately.
                for mb in range(mblk):
                    rs = mb * 128
                    for g in range(gap_split):
                        cs = n_out + g * gw
                        nc.sync.dma_start(out[rs:rs + 128, cs:cs + gw], zt[:])

                # Weights + activations + bias row in one DMA (one
                # semaphore).  Issued on the scalar HWDGE ring so it never
                # queues behind the gap stores on the sync ring.
                wx_sb = cpool.tile([128, kt, wcols], F32, tag="wx_sb")
                nc.scalar.dma_start(wx_sb[:], wx_r[:])

                # pred = x @ W.T + b, one 128-row block at a time.
                for mb in range(mblk):
                    rs = mb * 128
                    pt = ppool.tile([128, n_out], F32)
                    for n in range(nchunk):
                        ns = n * nw
                        ps = pspool.tile([128, nw], F32)
                        for k in range(kt):
                            nc.tensor.matmul(
                                ps[:],
                                wx_sb[:, k, n_out + rs:n_out + rs + 128],
                                wx_sb[:, k, ns:ns + nw],
                                start=(k == 0),
                                stop=(k == kt - 1),
                            )
                        # PSUM -> SBUF move (bias folded into matmul).
                        nc.vector.tensor_copy(pt[:, ns:ns + nw], ps[:])
                    # Band write: row t of this block goes to out[t, 0:n_out],
                    # which in flat space is the diagonal 65568*t + [0, 2048).
                    # Issued via SWDGE (gpsimd) whose lanes are otherwise
                    # idle: every instruction here may carry at most ONE sync
                    # wait, and on the sync ring this DMA would need a
                    # lane-FIFO wait on top of its DVE data wait.
                    nc.gpsimd.dma_start(out[rs:rs + 128, 0:n_out], pt[:])

    _split_multi_waits(nc)
    return nc


_CACHE = {}


def _get_nc():
    if "nc" not in _CACHE:
        _CACHE["nc"] = build_nc()
    return _CACHE["nc"]


def make_in_maps(x, W, b):
    """Host-side sharding: per-core packed input dicts."""
    d_pad = -(-(D_IN + 1) // 128) * 128  # 896
    x = x.astype(np.float32, copy=False)
    W = W.astype(np.float32, copy=False)
    b = b.astype(np.float32, copy=False)
    in_maps = []
    for c in range(N_CORES):
        bi, cc = divmod(c, CHUNKS)
        xs = x[bi, cc * ROWS:(cc + 1) * ROWS, :]
        wx_np = np.zeros((d_pad, N_OUT + ROWS), np.float32)
        wx_np[:D_IN, :N_OUT] = W.T
        wx_np[:D_IN, N_OUT:] = xs.T
        wx_np[D_IN, :N_OUT] = b
        wx_np[D_IN, N_OUT:] = 1.0
        in_maps.append({"wx": wx_np})
    return in_maps


def unshard(results):
    """Host-side unsharding: drop tail pad, rotate along S by 512*cc, place."""
    row_f = S * SPAN_DIM
    out = np.empty((B, S, S, SPAN_DIM), np.float32)
    for c in range(N_CORES):
        bi, cc = divmod(c, CHUNKS)
        buf = np.asarray(results[c]["out"])
        local = buf.reshape(-1)[:ROWS * row_f].reshape(ROWS, S, SPAN_DIM)
        sh = cc * ROWS
        blk = out[bi, sh:sh + ROWS]
        if sh:
            blk[:, sh:, :] = local[:, :S - sh, :]
            blk[:, :sh, :] = local[:, S - sh:, :]
        else:
            blk[:, :, :] = local
    return out


def kernel(x, W, b):
    x = np.asarray(x)
    W = np.asarray(W)
    b = np.asarray(b)
    nc = _get_nc()
    res = run_bass_kernel_spmd(nc, make_in_maps(x, W, b),
                               list(range(N_CORES)))
    return unshard(res.results)
